# revision 17
# baseline (speedup 1.0000x reference)
"""LoRA linear layer on 8 Trainium2 NeuronCores.

Computes y = x @ W^T + b + 2.0 * (x @ A^T) @ B^T for
x:[4,4096,1024], W:[1024,1024], b:[1024], A:[16,1024], B:[1024,16].

Host side folds the LoRA update into the weight (W_eff = W + 2*B@A, an exact
algebraic identity), so the device kernel is a single GEMM + bias. Sharding is
data-parallel over the 16384 tokens: each of the 8 cores computes a
[2048, 1024] output slice with replicated weights.

Device kernel (per core): y_c[m,o] = sum_d xT_c[d,m] * WeffT[d,o] + b[o]
  - xT_c  [1024, 2048] bf16 (host-transposed so the contraction dim d lands on
    SBUF partitions for both matmul operands; bf16 halves HBM traffic and runs
    at the same 1 col/cycle PE rate as f32r)
  - WeffT [1024, 1024] bf16, fully resident in SBUF
  - fp32 PSUM accumulation; bias broadcast to 128 partitions on host; fused
    add on the DVE during PSUM->SBUF eviction, output stored as bf16
  - no warmup matmuls: the first real matmuls run during the HAM cold window
    while the DMA ramp finishes, warming the clock with useful work
"""

import os

import numpy as np
from ml_dtypes import bfloat16

import concourse.mybir as mybir
import concourse.tile as tile
from concourse import bacc
from concourse.bass_utils import run_bass_kernel_spmd

N_CORES = 8
P = 128
D = 1024  # in_features (contraction)
O = 1024  # out_features
M_TOTAL = 4 * 4096  # tokens
M = M_TOTAL // N_CORES  # tokens per core
KO = D // P  # k-subtiles
SC = 512  # m super-chunk (DMA granularity)
SCALING = 2.0

# Set by test harnesses to capture profiling info; harmless otherwise.
TRACE = False
LAST_RESULT = None

_NC_CACHE = None


def _build_nc():
    f32 = mybir.dt.float32
    bf16 = mybir.dt.bfloat16

    nc = bacc.Bacc("TRN2", debug=False)
    xT = nc.dram_tensor("xT", [D, M], bf16, kind="ExternalInput")
    wT = nc.dram_tensor("wT", [D, O], bf16, kind="ExternalInput")
    bias = nc.dram_tensor("bias", [P, O], f32, kind="ExternalInput")
    y = nc.dram_tensor("y", [M, O], bf16, kind="ExternalOutput")

    xT_v = xT[:].rearrange("(ko p) m -> p ko m", p=P)  # [128, 8, 2048]
    wT_v = wT[:].rearrange("(ko p) o -> p ko o", p=P)  # [128, 8, 1024]
    y_v = y[:].rearrange("(mt p) o -> p mt o", p=P)  # [128, 16, 1024]

    n_sc = M // SC
    with tile.TileContext(nc) as tc:
        with (
            tc.tile_pool(name="wpool", bufs=1) as wpool,
            tc.tile_pool(name="bpool", bufs=1) as bpool,
            tc.tile_pool(name="xpool", bufs=16) as xpool,
            tc.tile_pool(name="opool", bufs=6) as opool,
            tc.tile_pool(name="psum", bufs=8, space="PSUM") as psum,
        ):
            # DMA queue throughput is limited by per-partition packet count
            # (~13-20 ns/packet per queue, size-independent up to 2 KiB), so
            # every tile is loaded with the widest possible per-partition
            # line: x in token-pair granules [128, 1024] (2 KiB lines) and W
            # as full [128, 1024] tiles (2 KiB lines). Each dma_start also
            # costs ~650 ns of descriptor gen on its issuing engine, so the
            # loads are spread across all three DMA-capable queues with each
            # queue's items ordered by consumption deadline: the first
            # matmul's two dependencies (x pair0/ko0 on sync, w0 on scalar)
            # start their descriptor gen in parallel when user code begins.
            xts = {}
            wt = [None] * KO

            def load_xp(pair, ko, eng):
                t = xpool.tile([P, 1024], bf16, tag="xt", name=f"xp{pair}_{ko}")
                eng.dma_start(t[:], xT_v[:, ko, pair * 1024 : (pair + 1) * 1024])
                xts[(pair, ko)] = t

            def load_w(ko, eng):
                t = wpool.tile([P, O], bf16, tag=f"w{ko}")
                eng.dma_start(t[:], wT_v[:, ko, :])
                wt[ko] = t

            # Interleave so each queue's early items match the matmul
            # stream's ko order.
            load_xp(0, 0, nc.sync)
            load_w(0, nc.scalar)
            load_w(1, nc.gpsimd)
            for ko in range(1, 6):
                load_xp(0, ko, nc.sync)
            load_w(2, nc.scalar)
            load_w(4, nc.scalar)
            load_w(6, nc.scalar)
            load_xp(0, 6, nc.scalar)
            load_xp(0, 7, nc.scalar)
            load_w(3, nc.gpsimd)
            load_w(5, nc.gpsimd)
            load_w(7, nc.gpsimd)
            bt = bpool.tile([P, O], f32)
            nc.gpsimd.dma_start(bt[:], bias[:])
            for ko in range(KO):
                load_xp(1, ko, nc.gpsimd)

            # Warmup: ~26 tiny N=128 matmuls (~107 ns each cold) fill the
            # ~2.8 us DMA ramp so the HAM activity window sees sustained PE
            # work from the moment user code starts -- the clock gate opens
            # (1.2 -> 2.4 GHz) during or right after the ramp instead of
            # 3.4-6.8 us into the real matmul stream. memset runs on the
            # vector engine, which has no other work during the ramp.
            n_warm = int(os.environ.get("KWARM", "26"))
            if n_warm:
                zt = wpool.tile([P, P], bf16, tag="warm")
                nc.vector.memset(zt[:], 0.0)
                wps = psum.tile([P, P], mybir.dt.float32, tag="ps", name="wps")
                for _ in range(n_warm):
                    nc.tensor.matmul(wps[:], zt[:], zt[:], start=True, stop=True)

            def x_slice(sc, ko, mt_i):
                off = (sc % 2) * SC + mt_i * P
                return xts[(sc // 2, ko)][:, off : off + P]

            def evict_half(ps, ot, half):
                nc.vector.tensor_tensor(
                    ot[:, half * 512 : (half + 1) * 512],
                    ps[:],
                    bt[:, half * 512 : (half + 1) * 512],
                    mybir.AluOpType.add,
                )

            MPC = SC // P  # m-tiles per super-chunk

            # Every super-chunk runs ko-outer: all four m-tiles accumulate
            # simultaneously across the 8 single-bank PSUM groups, so each W/x
            # slice is consumed as it lands during the ramp and the PE never
            # sits behind one large dependency. Evictions + stores are inlined
            # right behind each group's stop so PSUM slots recycle smoothly
            # into the next super-chunk.
            for sc in range(n_sc - 1):
                pss = [
                    [
                        psum.tile(
                            [P, 512], mybir.dt.float32, tag="ps", name=f"ps{sc}_{i}_{h}"
                        )
                        for h in range(2)
                    ]
                    for i in range(MPC)
                ]
                ots = [
                    opool.tile([P, O], bf16, tag="ot", name=f"ot{sc}_{i}")
                    for i in range(MPC)
                ]
                # half0-first within each ko group: the first matmuls of the
                # kernel depend only on x00 + w0h0 (the scalar queue's first
                # transfer), while the h1 weights ride the slower-starting
                # gpsimd queue and are consumed ~1.7 us later.
                for ko in range(KO):
                    last = ko == KO - 1
                    for half in range(2):
                        for mt_i in range(MPC):
                            mt = sc * MPC + mt_i
                            nc.tensor.matmul(
                                pss[mt_i][half][:],
                                x_slice(sc, ko, mt_i),
                                wt[ko][:, half * 512 : (half + 1) * 512],
                                start=ko == 0,
                                stop=last,
                            )
                            if last:
                                evict_half(pss[mt_i][half], ots[mt_i], half)
                                st_eng = nc.gpsimd if sc < 2 else nc.sync
                                st_eng.dma_start(
                                    y_v[:, mt, half * 512 : (half + 1) * 512],
                                    ots[mt_i][:, half * 512 : (half + 1) * 512],
                                )

            # Last super-chunk: mt-outer, so evictions and stores spread across
            # its whole span instead of piling up after the final matmul; the
            # very last m-tile runs its two output halves back to back so
            # half 0's eviction/store hides under half 1's matmuls.
            sc = n_sc - 1
            for mt_i in range(MPC):
                mt = sc * MPC + mt_i
                ot = opool.tile([P, O], bf16, tag="ot", name=f"otf{mt_i}")
                final = mt_i == MPC - 1
                if not final:
                    ph = [
                        psum.tile([P, 512], mybir.dt.float32, tag="ps", name=f"pl{h}")
                        for h in range(2)
                    ]
                    for ko in range(KO):
                        for half in range(2):
                            nc.tensor.matmul(
                                ph[half][:],
                                x_slice(sc, ko, mt_i),
                                wt[ko][:, half * 512 : (half + 1) * 512],
                                start=ko == 0,
                                stop=ko == KO - 1,
                            )
                    # Last super-chunk stores go on the sync/scalar hardware
                    # DGE queues (both idle by now) instead of gpsimd, whose
                    # software-DGE store queue drains slowly -- the postamble
                    # waits on every DMA, so a gpsimd backlog at the end gates
                    # the whole kernel.
                    for half, eng in ((0, nc.sync), (1, nc.scalar)):
                        evict_half(ph[half], ot, half)
                        eng.dma_start(
                            y_v[:, mt, half * 512 : (half + 1) * 512],
                            ot[:, half * 512 : (half + 1) * 512],
                        )
                else:
                    # The very last eviction + store sit on the kernel's
                    # critical path (nothing overlaps them), so the final
                    # output is produced in the smallest useful pieces: h0 as
                    # one N=512 group, then h1 as two N=256 accumulation
                    # groups so the last serial chain is a [128, 256] evict
                    # plus a store already split across the sync + scalar
                    # queues by partition range. gpsimd is deliberately
                    # avoided here: its software-DGE store queue drains
                    # slowly and the postamble waits on every DMA.
                    def split_store(lo, hi):
                        for eng, p0, p1 in (
                            (nc.sync, 0, 64),
                            (nc.scalar, 64, 128),
                        ):
                            eng.dma_start(
                                y_v[p0:p1, mt, lo:hi],
                                ot[p0:p1, lo:hi],
                            )

                    ps = psum.tile([P, 512], mybir.dt.float32, tag="ps", name="pf")
                    for ko in range(KO):
                        nc.tensor.matmul(
                            ps[:],
                            x_slice(sc, ko, mt_i),
                            wt[ko][:, 0:512],
                            start=ko == 0,
                            stop=ko == KO - 1,
                        )
                    evict_half(ps, ot, 0)
                    split_store(0, 512)
                    for q in range(2):
                        lo = 512 + q * 256
                        psq = psum.tile(
                            [P, 256], mybir.dt.float32, tag="ps", name=f"pfq{q}"
                        )
                        for ko in range(KO):
                            nc.tensor.matmul(
                                psq[:],
                                x_slice(sc, ko, mt_i),
                                wt[ko][:, lo : lo + 256],
                                start=ko == 0,
                                stop=ko == KO - 1,
                            )
                        nc.vector.tensor_tensor(
                            ot[:, lo : lo + 256],
                            psq[:],
                            bt[:, lo : lo + 256],
                            mybir.AluOpType.add,
                        )
                        split_store(lo, lo + 256)

    nc.compile()
    return nc


def _get_nc():
    global _NC_CACHE
    if _NC_CACHE is None:
        _NC_CACHE = _build_nc()
    return _NC_CACHE


def kernel(x, W, b, A, B):
    global LAST_RESULT
    x = np.ascontiguousarray(np.asarray(x, dtype=np.float32))
    W = np.asarray(W, dtype=np.float32)
    b = np.asarray(b, dtype=np.float32)
    A = np.asarray(A, dtype=np.float32)
    B = np.asarray(B, dtype=np.float32)
    assert x.shape == (4, 4096, D) and W.shape == (O, D)
    assert b.shape == (O,) and A.shape[1] == D and B.shape[0] == O

    # Fold the LoRA update into the weight: x@W^T + s*(x@A^T)@B^T = x@(W + s*B@A)^T
    Weff = (
        W.astype(np.float64) + SCALING * (B.astype(np.float64) @ A.astype(np.float64))
    ).astype(np.float32)
    WeffT = np.ascontiguousarray(Weff.T).astype(bfloat16)  # [D, O]
    bias_rep = np.ascontiguousarray(np.broadcast_to(b[None, :], (P, O)))

    xr = x.reshape(M_TOTAL, D)
    in_maps = []
    for c in range(N_CORES):
        xTc = np.ascontiguousarray(xr[c * M : (c + 1) * M].T).astype(bfloat16)  # [D, M]
        in_maps.append({"xT": xTc, "wT": WeffT, "bias": bias_rep})

    nc = _get_nc()
    res = run_bass_kernel_spmd(
        nc, in_maps, core_ids=list(range(N_CORES)), trace=TRACE
    )
    LAST_RESULT = res

    out = np.concatenate(
        [res.results[c]["y"].astype(np.float32) for c in range(N_CORES)], axis=0
    )
    return out.reshape(x.shape[0], x.shape[1], O)
